# revision 20
# baseline (speedup 1.0000x reference)
"""Trainium2 Bass kernel for DPMultiheadAttention (L=2048, B=2, E=1024, H=16).

Sharding: batch*head parallel across 8 cores. Core c handles batch c%2 and
heads [4*(c//2), 4*(c//2)+4). Each core computes q/k/v projections for its
256-feature slice, per-head attention, and a partial out-projection; the host
sums the per-batch partials.

Software-pipelined schedule (v2): the kernel is one long interleaved stream
instead of serial phases. The exp stream on the Scalar engine is the
second-longest resource (~136us busy) after PE (~164us), so:
  - xk/xq/xv are DMAed in parallel on the three DGE queues (scalar/sync/
    gpsimd) so K0/Q0 projections finish ~25us in and the first scores/exp
    start there, not at 80us.
  - Attention runs as 8 windows (qh, pair, head): scores -> exp -> ctx per
    128-key chunk j. Remaining projection work (V, K1/Q1) and the qh0
    out-projection are interleaved into the windows as PE filler so the PE
    queue never idles while the Scalar engine works through the exps.
  - PSUM: scores double-buffer (4 banks) + one ctx accumulator (2 banks) +
    a shared 2-buf pool for projections/out-proj (2 banks) = 8 banks.
  - Softmax denominators ride as a ones-column in the padded V operand;
    normalization (reciprocal spread over 4 partitions, DMA row-broadcast)
    happens per window off the critical path; out-projection per 128-token
    chunk follows once all four heads' ctx for that qh are normalized.
"""

import numpy as np

import concourse.bass as bass
import concourse.tile as tile
from concourse import mybir
from concourse.bass_utils import run_bass_kernel_spmd

L = 2048
B = 2
E = 1024
H = 16
D = 64
NCORES = 8
HPC = H // NCORES * B  # heads per core = 4
FL = HPC * D  # local feature slice = 256
P = 128

BF16 = mybir.dt.bfloat16
FP32 = mybir.dt.float32

TRACE = False
TRACE_KWARGS = {}
LAST_RESULTS = None


class PatchedTileContext(tile.TileContext):
    """This walrus build caps sync-wait slots per instruction at one; Tile's
    sem assigner freely attaches several. Split extra waits onto same-engine
    nops inserted just before the owning instruction."""

    MAX_WAITS = 1

    def _split_inst_waits(self, inst, out_list):
        si = getattr(inst, "sync_info", None)
        if si is not None and len(si.on_wait) > self.MAX_WAITS:
            waits = list(si.on_wait)
            keep = len(waits) - self.MAX_WAITS
            for i in range(0, keep, self.MAX_WAITS):
                out_list.append(
                    mybir.InstNoOp(
                        name=f"I-ws-{self.nc.next_id()}",
                        engine=inst.engine,
                        bass_nofuse=True,
                        sync_info=mybir.SyncInfo(
                            on_wait=waits[i : i + self.MAX_WAITS], on_update=[]
                        ),
                    )
                )
            inst.sync_info = mybir.SyncInfo(
                on_wait=waits[keep:], on_update=list(si.on_update)
            )
        out_list.append(inst)

    def _lower_ordered_insts(self, ordered):
        for insts in ordered.values():
            new_list = []
            for inst in insts:
                self._split_inst_waits(inst, new_list)
            insts[:] = new_list
        super()._lower_ordered_insts(ordered)

    def _drain_and_barrier(self, tick_clock, wait_clock):
        from bass_rust import SyncInfo
        from concourse.vector_clock import ScopedClock

        drain_inst = self.nc.sync.drain()
        wait_clock.add_sem_waits(
            drain_inst.ins, ScopedClock({None: tick_clock.global_clock})
        )
        si = drain_inst.ins.sync_info
        if si is not None and len(si.on_wait) > self.MAX_WAITS:
            waits = list(si.on_wait)
            drain_inst.ins.sync_info = SyncInfo(
                on_wait=waits[: self.MAX_WAITS], on_update=list(si.on_update)
            )
            for i in range(self.MAX_WAITS, len(waits), self.MAX_WAITS):
                nop = self.nc.sync.nop(nofuse=True)
                nop.ins.sync_info = SyncInfo(
                    on_wait=waits[i : i + self.MAX_WAITS], on_update=[]
                )

        self.nc.all_engine_barrier()
        assert self.sems is not None
        popped = self.nc._tile_sem_poison_stack.pop()
        assert popped is self._sem_poison
        self.nc.clear_and_free_semaphores(list(self.sems.allocated().values()))
        self.nc.all_engine_barrier()


def _ap3(ap, dims):
    return bass.AP(tensor=ap.tensor, offset=ap.offset, ap=dims)


def _bcast_ap(t):
    """DRAM 1-D tensor -> (128, len) partition-broadcast AP for DMA."""
    ap = t[:]
    return bass.AP(tensor=ap.tensor, offset=ap.offset, ap=[[0, P], *ap.ap])


KT = E // P  # 8 contraction tiles for projections
MT = FL // P  # 2 feature tiles (pairs)
NQ = L // 512  # 4 token chunks of 512
LT = L // P  # 16 token tiles of 128
EXPF = mybir.ActivationFunctionType.Exp


def build_nc():
    nc = bass.Bass()

    xq_d = [
        nc.declare_dram_parameter(f"xq{n}", [P, KT, 512], BF16, isOutput=False)
        for n in range(NQ)
    ]
    xk_d = [
        nc.declare_dram_parameter(f"xk{n}", [P, KT, 512], BF16, isOutput=False)
        for n in range(NQ)
    ]
    xv_d = [
        nc.declare_dram_parameter(f"xv{n}", [P, KT, 512], BF16, isOutput=False)
        for n in range(NQ)
    ]
    wq = nc.declare_dram_parameter("wq_t", [P, KT, FL], BF16, isOutput=False)
    wk = nc.declare_dram_parameter("wk_t", [P, KT, FL], BF16, isOutput=False)
    wv = nc.declare_dram_parameter("wv_t", [P, KT, FL], BF16, isOutput=False)
    wo = nc.declare_dram_parameter("wo_t", [P, MT, E], BF16, isOutput=False)
    bq = nc.declare_dram_parameter("bq", [FL], FP32, isOutput=False)
    bk = nc.declare_dram_parameter("bk", [FL], FP32, isOutput=False)
    bv = nc.declare_dram_parameter("bv", [FL], FP32, isOutput=False)
    bo = nc.declare_dram_parameter("bo", [E], BF16, isOutput=False)
    out = nc.declare_dram_parameter("out_p", [L, E], BF16, isOutput=True)

    with PatchedTileContext(nc) as tc:
        with (
            tc.tile_pool(name="singles", bufs=1) as singles,
            tc.tile_pool(name="pt", bufs=6) as pt_pool,
            tc.tile_pool(name="norm", bufs=2) as norm_pool,
            tc.tile_pool(name="outsb", bufs=3) as out_pool,
        ):
            # ---- activation-table preload: tiny exp before anything else ----
            dummy = singles.tile([1, 32], FP32, tag="dummy")
            nc.vector.memset(dummy[:], 1.0)
            nc.scalar.activation(dummy[:], dummy[:], EXPF)

            # ---- weights / biases ----
            wq_sb = singles.tile([P, KT, FL], BF16, tag="wq")
            wk_sb = singles.tile([P, KT, FL], BF16, tag="wk")
            wv_sb = singles.tile([P, KT, FL], BF16, tag="wv")
            wo_sb = singles.tile([P, MT, E], BF16, tag="wo")
            bq_sb = singles.tile([P, MT], FP32, tag="bq")
            bk_sb = singles.tile([P, MT], FP32, tag="bk")
            bv_sb = singles.tile([P, FL], FP32, tag="bv")
            mk_sb = singles.tile([P, P], BF16, tag="mk")
            bo_bf = singles.tile([P, E], BF16, tag="bo_bf")
            tr2_sb = singles.tile([P, 1024], BF16, tag="tr2")

            # ---- inputs: separate tiles per 512-token chunk so projection
            # matmuls gate on exactly the chunk they read ----
            xq_t = [
                singles.tile([P, KT, 512], BF16, tag=f"xq{n}", name=f"xq{n}")
                for n in range(NQ)
            ]
            xk_t = [
                singles.tile([P, KT, 512], BF16, tag=f"xk{n}", name=f"xk{n}")
                for n in range(NQ)
            ]
            xv_t = [
                singles.tile([P, KT, 512], BF16, tag=f"xv{n}", name=f"xv{n}")
                for n in range(NQ)
            ]

            # ---- persistent activations, dependency-granular ----
            # qtp[pair][qh]: Q^T zero-padded per head (head hh in rows
            # [64*hh, 64*hh+64)); kt[pair][nq]: K^T 512-token chunks.
            qtp_t = [
                [
                    singles.tile([P, 2, 1024], BF16, tag=f"qtp{p}{q}", name=f"qtp{p}{q}")
                    for q in range(2)
                ]
                for p in range(MT)
            ]
            kt_t = [
                [
                    singles.tile([P, 512], BF16, tag=f"kt{p}{n}", name=f"kt{p}{n}")
                    for n in range(NQ)
                ]
                for p in range(MT)
            ]
            v_t = [
                singles.tile([P, HPC, P], BF16, tag=f"v{j}", name=f"v{j}")
                for j in range(LT)
            ]
            ctx_t = [
                singles.tile([P, L], BF16, tag=f"ctx{p}", name=f"ctx{p}")
                for p in range(MT)
            ]

            nc.vector.memset(mk_sb[:], 0.0)
            nc.vector.memset(mk_sb[0:1, :], 1.0)
            nc.vector.memset(tr2_sb[:], 0.0)
            for p in range(MT):
                for q in range(2):
                    nc.vector.memset(qtp_t[p][q][D:P, 0, :], 0.0)
                    nc.vector.memset(qtp_t[p][q][0:D, 1, :], 0.0)
            for j in range(LT):
                nc.vector.memset(v_t[j][:, :, D : D + 1], 1.0)
                nc.vector.memset(v_t[j][:, :, D + 1 : P], 0.0)

            # ---- DMA waves on the sync DGE, gated so early-needed tensors
            # get full bandwidth. Each gate is a tiny DMA whose source is the
            # previous wave's last tile; its destination is a zero-pad cell
            # of a v tile (a legit-read location; the pad memset above is
            # ordered after it and restores the zeros).
            def dma_x(t, d, n):
                nc.sync.dma_start(t[n][:], d[n][:])

            gate_n = [0]

            def wave_gate(t, n):
                g = gate_n[0]
                gate_n[0] += 1
                nc.sync.dma_start(
                    v_t[15 - g][0:1, 0, P - 16 : P], t[n][0:1, 0, 0:16]
                )

            # wave 1: everything window 0's start needs
            nc.sync.dma_start(wk_sb[:], wk[:])
            nc.sync.dma_start(bk_sb[:], bk.rearrange("(o p) -> p o", p=P))
            nc.sync.dma_start(wq_sb[:], wq[:])
            nc.sync.dma_start(bq_sb[:], bq.rearrange("(o p) -> p o", p=P))
            dma_x(xk_t, xk_d, 0)
            dma_x(xq_t, xq_d, 0)
            dma_x(xq_t, xq_d, 1)
            wave_gate(xq_t, 1)

            def dma_x2(t, d, n):
                # two half-chunk transfers ride two DGE queues in parallel
                nc.sync.dma_start(t[n][:, 0:4, :], d[n][:, 0:4, :])
                nc.sync.dma_start(t[n][:, 4:8, :], d[n][:, 4:8, :])

            nc.sync.dma_start(wv_sb[:], wv[:])
            nc.sync.dma_start(bv_sb[:], _bcast_ap(bv))
            dma_x2(xv_t, xv_d, 0)
            dma_x2(xk_t, xk_d, 1)
            dma_x2(xv_t, xv_d, 1)
            dma_x2(xk_t, xk_d, 2)
            wave_gate(xk_t, 2)
            dma_x2(xk_t, xk_d, 3)
            dma_x2(xv_t, xv_d, 2)
            dma_x2(xv_t, xv_d, 3)
            wave_gate(xk_t, 3)
            dma_x(xq_t, xq_d, 2)
            dma_x(xq_t, xq_d, 3)
            nc.sync.dma_start(wo_sb[:], wo[:])
            nc.sync.dma_start(bo_bf[:], _bcast_ap(bo))

            with (
                tc.tile_pool(name="s_psum", bufs=2, space="PSUM") as s_pool,
                tc.tile_pool(name="c_psum", bufs=1, space="PSUM") as c_pool,
                tc.tile_pool(name="pa", bufs=2, space="PSUM") as pa_pool,
            ):

                def emit_proj(which, p, n):
                    """One 512-token chunk of a K or Q projection."""
                    w_sb, x_t, b_sb = (
                        (wk_sb, xk_t, bk_sb) if which == "k" else (wq_sb, xq_t, bq_sb)
                    )
                    ps = pa_pool.tile(
                        [P, 512], FP32, tag="pa", name=f"ps{which}{p}{n}"
                    )
                    for k in range(KT):
                        nc.tensor.matmul(
                            ps[:],
                            w_sb[:, k, bass.ts(p, P)],
                            x_t[n][:, k, :],
                            start=(k == 0),
                            stop=(k == KT - 1),
                        )
                    if which == "k":
                        nc.vector.tensor_scalar_add(
                            kt_t[p][n][:], ps[:], b_sb[:, p : p + 1]
                        )
                    else:
                        qh, half = n // 2, n % 2
                        nc.vector.tensor_scalar_add(
                            qtp_t[p][qh][0:D, 0, bass.ts(half, 512)],
                            ps[0:D],
                            b_sb[0:D, p : p + 1],
                        )
                        nc.vector.tensor_scalar_add(
                            qtp_t[p][qh][D:P, 1, bass.ts(half, 512)],
                            ps[D:P],
                            b_sb[D:P, p : p + 1],
                        )

                def emit_v(lt):
                    g = lt // 4
                    ps = pa_pool.tile([P, 512], FP32, tag="pa", name=f"psv{lt}")
                    for k in range(KT):
                        nc.tensor.matmul(
                            ps[:, :FL],
                            xv_t[g][:, k, bass.ts(lt % 4, P)],
                            wv_sb[:, k, :],
                            start=(k == 0),
                            stop=(k == KT - 1),
                        )
                    nc.vector.tensor_add(
                        v_t[lt][:, :, 0:D],
                        ps[:, :FL].rearrange("p (h d) -> p h d", d=D),
                        bv_sb.rearrange("p (h d) -> p h d", d=D),
                    )

                def emit_c(lt, tail=False):
                    osb = out_pool.tile([P, E], BF16, tag="osb", name=f"osb{lt}")
                    for nn in range(2):
                        ps = pa_pool.tile(
                            [P, 512], FP32, tag="pa", name=f"psc{lt}_{nn}"
                        )
                        for kt_i in range(MT):
                            nc.tensor.matmul(
                                ps[:],
                                ctx_t[kt_i][:, bass.ts(lt, P)],
                                wo_sb[:, kt_i, bass.ts(nn, 512)],
                                start=(kt_i == 0),
                                stop=(not tail and kt_i == MT - 1),
                            )
                        if tail:
                            # fold the bias in via a row-0-ones mask matmul so
                            # the PSUM drain is a pure copy, split across the
                            # (idle) Scalar engine and DVE
                            nc.tensor.matmul(
                                ps[:],
                                mk_sb[:],
                                bo_bf[:, bass.ts(nn, 512)],
                                start=False,
                                stop=True,
                            )
                            if nn == 0:
                                nc.scalar.copy(osb[:, bass.ts(nn, 512)], ps[:])
                            else:
                                nc.vector.tensor_copy(
                                    osb[:, bass.ts(nn, 512)], ps[:]
                                )
                        else:
                            nc.vector.tensor_add(
                                osb[:, bass.ts(nn, 512)],
                                ps[:],
                                bo_bf[:, bass.ts(nn, 512)],
                            )
                    nc.sync.dma_start(out[bass.ts(lt, P), :], osb[:])

                def emit_norm_front(qh, p, hh, cps):
                    """Drain cps (Scalar) and compute the reciprocal row:
                    DVE 32x32 block-transpose spreads the sums row over 32
                    partitions, reciprocal runs 32 lanes wide, transpose
                    back yields the full reciprocal row in partition 0."""
                    craw = norm_pool.tile(
                        [96, 1024], FP32, tag="craw", name=f"craw{qh}{p}{hh}"
                    )
                    nc.vector.tensor_copy(craw[0 : D + 1, :], cps[0 : D + 1, :])
                    tr = norm_pool.tile([32, 1024], FP32, tag="tr", name=f"tr{qh}{p}{hh}")
                    nc.vector.transpose(tr[:], craw[D : D + 32, :])
                    rv = _ap3(tr[:], [tr[:].ap[0], [32, 32]])
                    nc.vector.reciprocal(rv, rv)
                    trb = norm_pool.tile(
                        [32, 1024], FP32, tag="trb", name=f"trb{qh}{p}{hh}"
                    )
                    nc.vector.transpose(trb[:], tr[:])
                    nc.vector.tensor_copy(tr2_sb[0:1, :], trb[0:1, :])
                    return craw

                def emit_norm_back(qh, p, hh, fr):
                    """Broadcast the reciprocal row down 64 partitions with a
                    full-contraction mask matmul (row 0 of mk is ones, the
                    rest zeros null out the garbage rows of tr2), then scale
                    ctx."""
                    craw = fr
                    for nn in range(2):
                        rbp = pa_pool.tile(
                            [D, 512], FP32, tag="pa", name=f"rb{qh}{p}{hh}_{nn}"
                        )
                        nc.tensor.matmul(
                            rbp[:],
                            mk_sb[:, 0:D],
                            tr2_sb[:, bass.ts(nn, 512)],
                            start=True,
                            stop=True,
                        )
                        nc.vector.tensor_mul(
                            ctx_t[p][
                                D * hh : D * hh + D,
                                bass.ds(qh * 1024 + nn * 512, 512),
                            ],
                            craw[0:D, bass.ts(nn, 512)],
                            rbp[:],
                        )

                def emit_window(qh, p, hh, fillers_by_j):
                    head = 2 * p + hh
                    cps = c_pool.tile(
                        [P, 1024], FP32, tag="c", name=f"cps{qh}{p}{hh}"
                    )
                    for j in range(LT):
                        for f in fillers_by_j.get(j, ()):
                            if getattr(f, "pre", False):
                                f()
                        sps = s_pool.tile(
                            [P, 1024], FP32, tag="s", name=f"sps{qh}{p}{hh}_{j}"
                        )
                        for nn in range(2):
                            nc.tensor.matmul(
                                sps[:, bass.ts(nn, 512)],
                                kt_t[p][j // 4][:, bass.ts(j % 4, P)],
                                qtp_t[p][qh][:, hh, bass.ts(nn, 512)],
                                start=True,
                                stop=True,
                            )
                        ptile = pt_pool.tile(
                            [P, 1024], BF16, tag="pt", name=f"pt{qh}{p}{hh}_{j}"
                        )
                        nc.scalar.activation(ptile[:], sps[:], EXPF)
                        for f in fillers_by_j.get(j, ()):
                            if not getattr(f, "pre", False):
                                f()
                        for nn in range(2):
                            nc.tensor.matmul(
                                cps[:, bass.ts(nn, 512)],
                                v_t[j][:, head, :],
                                ptile[:, bass.ts(nn, 512)],
                                start=(j == 0),
                                stop=(j == LT - 1),
                            )
                    return emit_norm_front(qh, p, hh, cps)

                # K0n0, Q0n0, Q0n1 ahead of window 0 (they gate its scores)
                emit_proj("k", 0, 0)
                emit_proj("q", 0, 0)
                emit_proj("q", 0, 1)

                def F(*fs):
                    return list(fs)

                def NB(qh, p, hh, fr):
                    return lambda: emit_norm_back(qh, p, hh, fr)

                w0 = {j: [lambda lt=j: emit_v(lt)] for j in range(LT)}
                for m in (1, 2, 3):
                    kf = lambda n=m: emit_proj("k", 0, n)
                    kf.pre = True
                    w0[4 * m].insert(0, kf)
                f000 = emit_window(0, 0, 0, w0)
                f001 = emit_window(0, 0, 1, {
                    2: F(NB(0, 0, 0, f000)),
                    5: F(lambda: emit_proj("q", 0, 2)),
                    10: F(lambda: emit_proj("q", 0, 3)),
                })
                f100 = emit_window(1, 0, 0, {
                    2: F(NB(0, 0, 1, f001)),
                    4: F(lambda: emit_proj("k", 1, 0)),
                    7: F(lambda: emit_proj("k", 1, 1)),
                    10: F(lambda: emit_proj("k", 1, 2)),
                    13: F(lambda: emit_proj("k", 1, 3)),
                })
                f101 = emit_window(1, 0, 1, {
                    2: F(NB(1, 0, 0, f100)),
                    5: F(lambda: emit_proj("q", 1, 0)),
                    10: F(lambda: emit_proj("q", 1, 1)),
                })
                f010 = emit_window(0, 1, 0, {
                    2: F(NB(1, 0, 1, f101)),
                    5: F(lambda: emit_proj("q", 1, 2)),
                    10: F(lambda: emit_proj("q", 1, 3)),
                })
                f011 = emit_window(0, 1, 1, {2: F(NB(0, 1, 0, f010))})
                f110 = emit_window(1, 1, 0, {
                    2: F(NB(0, 1, 1, f011)),
                    **{j: F(lambda lt=(j - 4) // 3: emit_c(lt))
                       for j in range(4, 16, 3)},
                })
                w7 = {2: F(NB(1, 1, 0, f110)),
                      **{j: F(lambda lt=(j - 4) // 3 + 4: emit_c(lt))
                         for j in range(4, 16, 3)}}
                f111 = emit_window(1, 1, 1, w7)
                emit_norm_back(1, 1, 1, f111)
                for lt in range(8, LT):
                    emit_c(lt, tail=True)

    return nc


_NC = None


def _get_nc():
    global _NC
    if _NC is None:
        _NC = build_nc()
    return _NC


def kernel(query, key, value, w_in, b_in, w_out, b_out):
    import ml_dtypes

    bf16 = ml_dtypes.bfloat16
    query = np.asarray(query, dtype=np.float32)
    key = np.asarray(key, dtype=np.float32)
    value = np.asarray(value, dtype=np.float32)
    w_in = np.asarray(w_in, dtype=np.float32)
    b_in = np.asarray(b_in, dtype=np.float32)
    w_out = np.asarray(w_out, dtype=np.float32)
    b_out = np.asarray(b_out, dtype=np.float32)

    scale = float(D) ** -0.5
    in_maps = []
    for c in range(NCORES):
        b = c % 2
        g = c // 2
        sl = slice(FL * g, FL * (g + 1))
        wq = w_in[0 * E : 1 * E][sl] * scale  # (256, 1024)
        wk = w_in[1 * E : 2 * E][sl]
        wv = w_in[2 * E : 3 * E][sl]
        def chunks(x_lbe):
            # (L, B, E) batch slice -> 4 chunk tensors [P, KT, 512] bf16,
            # x_t[p, o, m] = x[m + 512n, b, p + 128o]
            xt = x_lbe[:, b, :].T.reshape(KT, P, NQ, 512).transpose(1, 0, 2, 3)
            return {
                n: np.ascontiguousarray(xt[:, :, n, :]).astype(bf16)
                for n in range(NQ)
            }

        def warr(w):
            # (FL, E) -> [P, KT, FL]: w_t[p, o, f] = w[f, p + 128o]
            return np.ascontiguousarray(
                w.T.reshape(KT, P, FL).transpose(1, 0, 2)
            ).astype(bf16)

        xqc, xkc, xvc = chunks(query), chunks(key), chunks(value)
        wo_arr = np.ascontiguousarray(
            w_out[:, sl].T.reshape(MT, P, E).transpose(1, 0, 2)
        ).astype(bf16)
        in_maps.append(
            {
                **{f"xq{n}": xqc[n] for n in range(NQ)},
                **{f"xk{n}": xkc[n] for n in range(NQ)},
                **{f"xv{n}": xvc[n] for n in range(NQ)},
                "wq_t": warr(wq),
                "wk_t": warr(wk),
                "wv_t": warr(wv),
                "wo_t": wo_arr,
                "bq": np.ascontiguousarray(b_in[0 * E : 1 * E][sl] * scale),
                "bk": np.ascontiguousarray(b_in[1 * E : 2 * E][sl]),
                "bv": np.ascontiguousarray(b_in[2 * E : 3 * E][sl]),
                "bo": (b_out if c < 2 else np.zeros_like(b_out)).astype(bf16),
            }
        )

    nc = _get_nc()
    res = run_bass_kernel_spmd(
        nc, in_maps, list(range(NCORES)), trace=TRACE, **TRACE_KWARGS
    )
    global LAST_RESULTS
    LAST_RESULTS = res

    out = np.zeros((L, B, E), dtype=np.float32)
    for c in range(NCORES):
        out[:, c % 2, :] += np.asarray(res.results[c]["out_p"], dtype=np.float32)
    return out
